# revision 19
# baseline (speedup 1.0000x reference)
"""BiasAndSum Trainium2 kernel.

Reference semantics (xs: [T, 1, D] f32):
    ys    = xs[:, 0, :] + 1              # [T, D]
    carry = sum_t (xs[t] + 1)            # [1, D]

Strategy: shard T across 8 NeuronCores (T_SHARD = T/8 rows each). Per core,
stream 32 tiles of [128, D]:
  - load x tile (DMA on the SP queue, 10 of 32 on the ACT queue)
  - DVE computes y = x + 1 in f32 (exact) for the ys store
  - DVE pair-adds raw x tiles (z = x_a + x_b, f32); PE column-sums the 16
    z tiles via a ones[128,1] stationary matmul into PSUM (partition-axis
    reduction, fp32 accumulate); the +1-bias contribution (T_SHARD per
    column) is folded in when PSUM is read back
  - store y tile (DMA on the Pool queue, 10 of 32 on the ACT queue)
DMA transfer time is charged to the issuing engine queue, so loads and
stores are spread across the three DMA-capable queues (SP/ACT/Pool) to
balance them. Host concatenates ys shards and sums the 8 carry partials.
"""

import numpy as np

T, D = 32768, 2048
N_CORES = 8
T_SHARD = T // N_CORES  # 4096 rows per core
P = 128                 # SBUF partitions per tile
N_TILES = T_SHARD // P  # 32 tiles per core
MM_N = 512              # one PSUM bank of f32 per matmul output

# DMA issue queues are limited to SP (sync), Activation, and Pool (gpsimd).
# Per-DMA transfer time is charged to the issuing queue, so spread the 64
# tile DMAs: loads on SP with 10 on ACT, stores on Pool with 10 on ACT.
LOAD_ACT = {1, 4, 7, 10, 13, 16, 19, 22, 25, 28}
STORE_ACT = {2, 5, 8, 11, 14, 17, 20, 23, 26, 29}

_compiled_nc = None


def _build_nc():
    import concourse.bacc as bacc
    import concourse.mybir as mybir
    import concourse.tile as tile

    f32 = mybir.dt.float32

    nc = bacc.Bacc(
        "TRN2",
        target_bir_lowering=False,
        debug=False,
        enable_asserts=False,
        num_devices=N_CORES,
    )
    x = nc.dram_tensor("x", [T_SHARD, D], f32, kind="ExternalInput").ap()
    ys = nc.dram_tensor("ys", [T_SHARD, D], f32, kind="ExternalOutput").ap()
    carry = nc.dram_tensor("carry", [1, D], f32, kind="ExternalOutput").ap()

    n_banks = D // MM_N
    with tile.TileContext(nc) as tc:
        with (
            tc.tile_pool(name="x_pool", bufs=6) as xpool,
            tc.tile_pool(name="y_pool", bufs=6) as ypool,
            tc.tile_pool(name="z_pool", bufs=3) as zpool,
            tc.tile_pool(name="const_pool", bufs=1) as cpool,
            tc.tile_pool(name="psum_pool", bufs=1, space="PSUM") as ppool,
        ):
            ones = cpool.tile([P, 1], f32, tag="ones", name="ones")
            nc.vector.memset(ones[:], 1.0)

            accs = []
            for j in range(n_banks):
                acc = ppool.tile([1, MM_N], f32, tag=f"acc{j}", name=f"acc{j}")
                accs.append(acc)

            prev_xt = None
            for i in range(N_TILES):
                rows = slice(i * P, (i + 1) * P)

                xt = xpool.tile([P, D], f32)
                load_eng = nc.scalar if i in LOAD_ACT else nc.sync
                load_eng.dma_start(out=xt[:], in_=x[rows, :])

                # Exact f32 y = x + 1 for the ys store.
                yt = ypool.tile([P, D], f32)
                nc.vector.tensor_scalar_add(out=yt[:], in0=xt[:], scalar1=1.0)

                store_eng = nc.scalar if i in STORE_ACT else nc.gpsimd
                store_eng.dma_start(out=ys[rows, :], in_=yt[:])

                if i % 2 == 0:
                    prev_xt = xt
                    continue

                # Pair-add raw x tiles in f32, then PE column-sums z; this
                # halves the (4 cycles/row) fp32 matmul work vs summing
                # every tile while keeping the carry path fully fp32.
                z = zpool.tile([P, D], f32)
                nc.vector.tensor_add(out=z[:], in0=prev_xt[:], in1=xt[:])

                p = i // 2
                for j in range(n_banks):
                    nc.tensor.matmul(
                        accs[j][:],
                        ones[:],
                        z[:, j * MM_N:(j + 1) * MM_N],
                        start=(p == 0),
                        stop=(p == N_TILES // 2 - 1),
                    )

            res = cpool.tile([1, D], f32, tag="res", name="res")
            for j in range(n_banks):
                # PSUM -> SBUF, folding in the +1-bias contribution of the
                # T_SHARD rows this core summed.
                nc.vector.tensor_scalar_add(
                    out=res[:, j * MM_N:(j + 1) * MM_N],
                    in0=accs[j][:],
                    scalar1=float(T_SHARD),
                )
            nc.gpsimd.dma_start(out=carry[:], in_=res[:])

    nc.compile()
    return nc


def kernel(xs):
    global _compiled_nc
    from concourse.bass_utils import run_bass_kernel_spmd

    xs = np.ascontiguousarray(np.asarray(xs, dtype=np.float32)).reshape(T, D)
    if _compiled_nc is None:
        _compiled_nc = _build_nc()

    in_maps = [{"x": xs[c * T_SHARD:(c + 1) * T_SHARD]} for c in range(N_CORES)]
    results = run_bass_kernel_spmd(_compiled_nc, in_maps, list(range(N_CORES))).results

    ys_full = np.concatenate([r["ys"] for r in results], axis=0)
    carry = np.zeros((1, D), dtype=np.float32)
    for r in results:
        carry += r["carry"]
    return ys_full, carry


# revision 21
# speedup vs baseline: 1.0003x; 1.0003x over previous
"""BiasAndSum Trainium2 kernel.

Reference semantics (xs: [T, 1, D] f32):
    ys    = xs[:, 0, :] + 1              # [T, D]
    carry = sum_t (xs[t] + 1)            # [1, D]

Strategy: shard T across 8 NeuronCores (T_SHARD = T/8 rows each). Per core,
stream 32 tiles of [128, D]:
  - load x tile (DMA on the SP queue, 10 of 32 on the ACT queue)
  - DVE computes y = x + 1 in f32 (exact) for the ys store
  - DVE pair-adds raw x tiles (z = x_a + x_b, f32); PE column-sums the 16
    z tiles via a ones[128,1] stationary matmul into PSUM (partition-axis
    reduction, fp32 accumulate); the +1-bias contribution (T_SHARD per
    column) is folded in when PSUM is read back
  - store y tile (DMA on the Pool queue, 10 of 32 on the ACT queue)
DMA transfer time is charged to the issuing engine queue, so loads and
stores are spread across the three DMA-capable queues (SP/ACT/Pool) to
balance them. Host concatenates ys shards and sums the 8 carry partials.
"""

import numpy as np

T, D = 32768, 2048
N_CORES = 8
T_SHARD = T // N_CORES  # 4096 rows per core
P = 128                 # SBUF partitions per tile
N_TILES = T_SHARD // P  # 32 tiles per core
MM_N = 512              # one PSUM bank of f32 per matmul output

# DMA issue queues are limited to SP (sync), Activation, and Pool (gpsimd).
# Per-DMA transfer time is charged to the issuing queue, so interleave both
# loads and stores across all three queues, slightly under-weighting ACT
# (which also runs two of the +1 ops): SP 22 / ACT 20 / Pool 22 DMAs.


def _weighted_rr(counts):
    """Spread engine indices with the given counts roughly uniformly."""
    total = sum(counts)
    credits = [0.0] * len(counts)
    out = []
    for _ in range(total):
        for k in range(len(counts)):
            credits[k] += counts[k] / total
        pick = max(range(len(counts)), key=lambda k: credits[k])
        credits[pick] -= 1.0
        out.append(pick)
    return out


LOAD_Q = _weighted_rr([11, 10, 11])   # SP, ACT, Pool per load tile
STORE_Q = _weighted_rr([11, 10, 11])  # SP, ACT, Pool per store tile
ACT_ADD = {13, 27}                    # tiles whose +1 runs on ACT instead of DVE

_compiled_nc = None


def _build_nc():
    import concourse.bacc as bacc
    import concourse.mybir as mybir
    import concourse.tile as tile

    f32 = mybir.dt.float32

    nc = bacc.Bacc(
        "TRN2",
        target_bir_lowering=False,
        debug=False,
        enable_asserts=False,
        num_devices=N_CORES,
    )
    x = nc.dram_tensor("x", [T_SHARD, D], f32, kind="ExternalInput").ap()
    ys = nc.dram_tensor("ys", [T_SHARD, D], f32, kind="ExternalOutput").ap()
    carry = nc.dram_tensor("carry", [1, D], f32, kind="ExternalOutput").ap()

    n_banks = D // MM_N
    with tile.TileContext(nc) as tc:
        with (
            tc.tile_pool(name="x_pool", bufs=6) as xpool,
            tc.tile_pool(name="y_pool", bufs=6) as ypool,
            tc.tile_pool(name="z_pool", bufs=3) as zpool,
            tc.tile_pool(name="const_pool", bufs=1) as cpool,
            tc.tile_pool(name="psum_pool", bufs=1, space="PSUM") as ppool,
        ):
            ones = cpool.tile([P, 1], f32, tag="ones", name="ones")
            nc.vector.memset(ones[:], 1.0)

            accs = []
            for j in range(n_banks):
                acc = ppool.tile([1, MM_N], f32, tag=f"acc{j}", name=f"acc{j}")
                accs.append(acc)

            engines = [nc.sync, nc.scalar, nc.gpsimd]
            prev_xt = None
            for i in range(N_TILES):
                rows = slice(i * P, (i + 1) * P)
                load_eng = engines[LOAD_Q[i]]
                store_eng = engines[STORE_Q[i]]
                # First/last tile run in two column chunks so the pipeline
                # fills (first +1 starts sooner) and drains (last store
                # overlaps the last +1) faster.
                n_chunks = 2 if i in (0, N_TILES - 1) else 1
                w = D // n_chunks

                xt = xpool.tile([P, D], f32)
                yt = ypool.tile([P, D], f32)
                for c in range(n_chunks):
                    cols = slice(c * w, (c + 1) * w)
                    load_eng.dma_start(out=xt[:, cols], in_=x[rows, cols])
                    # Exact f32 y = x + 1 for the ys store.
                    if i in ACT_ADD:
                        nc.scalar.add(yt[:, cols], xt[:, cols], 1.0)
                    else:
                        nc.vector.tensor_scalar_add(
                            out=yt[:, cols], in0=xt[:, cols], scalar1=1.0
                        )
                    store_eng.dma_start(out=ys[rows, cols], in_=yt[:, cols])

                if i % 2 == 0:
                    prev_xt = xt
                    continue

                # Pair-add raw x tiles in f32, then PE column-sums z; this
                # halves the (4 cycles/row) fp32 matmul work vs summing
                # every tile while keeping the carry path fully fp32.
                z = zpool.tile([P, D], f32)
                nc.vector.tensor_add(out=z[:], in0=prev_xt[:], in1=xt[:])

                p = i // 2
                for j in range(n_banks):
                    nc.tensor.matmul(
                        accs[j][:],
                        ones[:],
                        z[:, j * MM_N:(j + 1) * MM_N],
                        start=(p == 0),
                        stop=(p == N_TILES // 2 - 1),
                    )

            res = cpool.tile([1, D], f32, tag="res", name="res")
            for j in range(n_banks):
                # PSUM -> SBUF, folding in the +1-bias contribution of the
                # T_SHARD rows this core summed.
                nc.vector.tensor_scalar_add(
                    out=res[:, j * MM_N:(j + 1) * MM_N],
                    in0=accs[j][:],
                    scalar1=float(T_SHARD),
                )
            nc.gpsimd.dma_start(out=carry[:], in_=res[:])

    nc.compile()
    return nc


def kernel(xs):
    global _compiled_nc
    from concourse.bass_utils import run_bass_kernel_spmd

    xs = np.ascontiguousarray(np.asarray(xs, dtype=np.float32)).reshape(T, D)
    if _compiled_nc is None:
        _compiled_nc = _build_nc()

    in_maps = [{"x": xs[c * T_SHARD:(c + 1) * T_SHARD]} for c in range(N_CORES)]
    results = run_bass_kernel_spmd(_compiled_nc, in_maps, list(range(N_CORES))).results

    ys_full = np.concatenate([r["ys"] for r in results], axis=0)
    carry = np.zeros((1, D), dtype=np.float32)
    for r in results:
        carry += r["carry"]
    return ys_full, carry


# revision 51
# speedup vs baseline: 1.1461x; 1.1458x over previous
"""BiasAndSum Trainium2 kernel.

Reference semantics (xs: [T, 1, D] f32):
    ys    = xs[:, 0, :] + 1              # [T, D]
    carry = sum_t (xs[t] + 1)            # [1, D]

Strategy: shard T across 8 NeuronCores (T_SHARD = T/8 rows each). Per core,
stream 32 tiles of [128, D] with loads software-pipelined 6 tiles ahead:
  - DVE computes y = x + 1 in f32 (exact) for the ys store
  - the carry column-sum (a partition-axis reduction) runs on the PE as a
    ones[128, 1] stationary matmul into PSUM (fp32 accumulate); most pairs
    of x tiles are pre-added on DVE (halves the 4-cycles/row fp32 matmul
    work), a few early pairs are matmul'd directly so the PE ramps
    immediately, and the final pair is reduced in interleaved half tiles
    so the closing stop-matmuls drain alongside the stores; the +1-bias
    contribution (T_SHARD per column) is folded in when PSUM is read back
  - loads and stores are interleaved across the three DMA-capable issue
    queues (SP/ACT/Pool) so their transfer time is balanced
  - the first tile runs in quarter tiles and the last in half tiles to
    shorten pipeline fill/drain
Host concatenates ys shards and sums the 8 carry partials.
"""

import numpy as np

T, D = 32768, 2048
N_CORES = 8
T_SHARD = T // N_CORES  # 4096 rows per core
P = 128                 # SBUF partitions per tile
N_TILES = T_SHARD // P  # 32 tiles per core
MM_N = 512              # one PSUM bank of f32 per matmul output

# DMA issue queues are limited to SP (sync), Activation, and Pool (gpsimd).
# Per-DMA transfer time is charged to the issuing queue, so interleave both
# loads and stores across all three queues, slightly under-weighting ACT
# (which also runs two of the +1 ops): SP 22 / ACT 20 / Pool 22 DMAs.


def _weighted_rr(counts):
    """Spread engine indices with the given counts roughly uniformly."""
    total = sum(counts)
    credits = [0.0] * len(counts)
    out = []
    for _ in range(total):
        for k in range(len(counts)):
            credits[k] += counts[k] / total
        pick = max(range(len(counts)), key=lambda k: credits[k])
        credits[pick] -= 1.0
        out.append(pick)
    return out


LOAD_Q = _weighted_rr([11, 10, 11])   # SP, ACT, Pool per load tile
# Stores: tiles 10 and 20 are split in half across two queues so each queue
# carries ~21.5 MiB instead of 22 (the 64 transfers don't divide by 3); the
# remaining 30 stores rotate evenly.
SPLIT_STORE = {10: (1, 0), 20: (1, 2)}  # tile -> (engine for lo half, hi half)
_rot = _weighted_rr([10, 10, 10])
STORE_Q = []
_k = 0
for _i in range(32):
    if _i in SPLIT_STORE:
        STORE_Q.append(None)
    else:
        STORE_Q.append(_rot[_k])
        _k += 1
ACT_ADD = set()                       # tiles whose +1 runs on ACT instead of DVE
# Pairs whose column-sum skips the DVE pair-add and matmuls both x tiles
# directly on the PE (PE has slack; chosen at the start so PE ramps early).
DIRECT_PAIRS = {0, 1, 2}

_compiled_nc = None


def _build_nc():
    import concourse.bacc as bacc
    import concourse.mybir as mybir
    import concourse.tile as tile

    f32 = mybir.dt.float32

    nc = bacc.Bacc(
        "TRN2",
        target_bir_lowering=False,
        debug=False,
        enable_asserts=False,
        num_devices=N_CORES,
    )
    x = nc.dram_tensor("x", [T_SHARD, D], f32, kind="ExternalInput").ap()
    ys = nc.dram_tensor("ys", [T_SHARD, D], f32, kind="ExternalOutput").ap()
    carry = nc.dram_tensor("carry", [1, D], f32, kind="ExternalOutput").ap()

    n_banks = D // MM_N
    with tile.TileContext(nc) as tc:
        with (
            tc.tile_pool(name="x_pool", bufs=8) as xpool,
            tc.tile_pool(name="y_pool", bufs=8) as ypool,
            tc.tile_pool(name="z_pool", bufs=4) as zpool,
            tc.tile_pool(name="const_pool", bufs=1) as cpool,
            tc.tile_pool(name="psum_pool", bufs=1, space="PSUM") as ppool,
        ):
            ones = cpool.tile([P, 1], f32, tag="ones", name="ones")
            nc.vector.memset(ones[:], 1.0)
            # f32r twin for the pre-added z tiles: DVE produces z directly
            # as float32r, which runs the PE at 1 cycle/row instead of 4.
            f32r = mybir.dt.float32r
            ones_r = cpool.tile([P, 1], f32r, tag="ones_r", name="ones_r")
            nc.vector.memset(ones_r[:].bitcast(f32), 1.0)
            # Bias tile holding the +1 contribution of this core's T_SHARD
            # rows, applied when PSUM is read back on the scalar engine.
            b_shard = cpool.tile([1, 1], f32, tag="b_shard", name="b_shard")
            nc.vector.memset(b_shard[:], float(T_SHARD))

            accs = []
            for j in range(n_banks):
                acc = ppool.tile([1, MM_N], f32, tag=f"acc{j}", name=f"acc{j}")
                accs.append(acc)

            engines = [nc.sync, nc.scalar, nc.gpsimd]
            # Software-pipeline program order: issue the load for tile
            # i+LOOKAHEAD before tile i's compute+store, so each DMA queue's
            # FIFO keeps load supply ahead of the (non-gating) stores.
            LOOKAHEAD = 3
            xts = {}

            def issue_load(i):
                rows = slice(i * P, (i + 1) * P)
                if i == 0:
                    # Dependencies are tracked per tile, so the first tile's
                    # quarters must be separate tiles for the first +1 to
                    # start after only a quarter transfer (pipeline fill).
                    quarters = []
                    for c in range(4):
                        cols = slice(c * (D // 4), (c + 1) * (D // 4))
                        xh = xpool.tile(
                            [P, D // 4], f32, name=f"x0h{c}",
                            tag=f"x0h{c}", bufs=1,
                        )
                        engines[LOAD_Q[i]].dma_start(out=xh[:], in_=x[rows, cols])
                        quarters.append(xh)
                    xts[i] = quarters
                    return
                if i == N_TILES - 1:
                    # Last tile in separate half tiles so the closing
                    # reduction and +1 start after half a transfer
                    # (pipeline drain).
                    halves = []
                    for c in range(2):
                        cols = slice(c * (D // 2), (c + 1) * (D // 2))
                        xh = xpool.tile(
                            [P, D // 2], f32, name=f"xLh{c}",
                            tag=f"xLh{c}", bufs=1,
                        )
                        engines[LOAD_Q[i]].dma_start(out=xh[:], in_=x[rows, cols])
                        halves.append(xh)
                    xts[i] = halves
                    return
                if i == N_TILES - 1:
                    # Last tile in separate half tiles so the closing
                    # reduction and +1 start after half a transfer
                    # (pipeline drain).
                    halves = []
                    for c in range(2):
                        cols = slice(c * (D // 2), (c + 1) * (D // 2))
                        xh = xpool.tile(
                            [P, D // 2], f32, name=f"xLh{c}",
                            tag=f"xLh{c}", bufs=1,
                        )
                        engines[LOAD_Q[i]].dma_start(out=xh[:], in_=x[rows, cols])
                        halves.append(xh)
                    xts[i] = halves
                    return
                xt = xpool.tile([P, D], f32, name=f"xt{i}", tag="xt")
                engines[LOAD_Q[i]].dma_start(out=xt[:], in_=x[rows, :])
                xts[i] = xt

            for i in range(LOOKAHEAD):
                issue_load(i)

            for i in range(N_TILES):
                if i + LOOKAHEAD < N_TILES:
                    issue_load(i + LOOKAHEAD)
                rows = slice(i * P, (i + 1) * P)
                xt = xts[i]
                store_eng = None if STORE_Q[i] is None else engines[STORE_Q[i]]

                if i == 0:
                    # Separate quarter tiles (see issue_load): +1 and store
                    # per quarter so the pipeline fills as data lands.
                    for c, xh in enumerate(xt):
                        cols = slice(c * (D // 4), (c + 1) * (D // 4))
                        yh = ypool.tile(
                            [P, D // 4], f32, name=f"y0h{c}",
                            tag=f"y0h{c}", bufs=1,
                        )
                        nc.vector.tensor_scalar_add(
                            out=yh[:], in0=xh[:], scalar1=1.0
                        )
                        store_eng.dma_start(out=ys[rows, cols], in_=yh[:])
                    continue

                if i == N_TILES - 1:
                    # Drain stage: the last tile arrives as two half tiles;
                    # per half, DVE immediately pair-adds with the matching
                    # slice of the previous tile (feeding the closing
                    # stop-matmuls) while ACT computes the +1 for the store.
                    # Carry stays fully fp32.
                    for c, xh in enumerate(xt):
                        cols = slice(c * (D // 2), (c + 1) * (D // 2))
                        zh = zpool.tile(
                            [P, D // 2], f32r, name=f"zLh{c}",
                            tag=f"zLh{c}", bufs=1,
                        )
                        nc.vector.tensor_add(
                            out=zh[:], in0=xts[i - 1][:, cols], in1=xh[:]
                        )
                        yh = ypool.tile(
                            [P, D // 2], f32, name=f"yLh{c}",
                            tag=f"yLh{c}", bufs=1,
                        )
                        nc.scalar.add(yh[:], xh[:], 1.0)
                        store_eng.dma_start(out=ys[rows, cols], in_=yh[:])
                        for jj in range(n_banks // 2):
                            j = c * (n_banks // 2) + jj
                            nc.tensor.matmul(
                                accs[j][:],
                                ones_r[:],
                                zh[:, jj * MM_N:(jj + 1) * MM_N],
                                start=False,
                                stop=True,
                            )
                    del xts[i - 1], xts[i]
                    continue
                else:
                    yt = ypool.tile([P, D], f32)
                    # Exact f32 y = x + 1 for the ys store.
                    if i in ACT_ADD:
                        nc.scalar.add(yt[:], xt[:], 1.0)
                    else:
                        nc.vector.tensor_scalar_add(
                            out=yt[:], in0=xt[:], scalar1=1.0
                        )
                    if i in SPLIT_STORE:
                        e0, e1 = SPLIT_STORE[i]
                        h = D // 2
                        engines[e0].dma_start(
                            out=ys[rows, 0:h], in_=yt[:, 0:h]
                        )
                        engines[e1].dma_start(
                            out=ys[rows, h:D], in_=yt[:, h:D]
                        )
                    else:
                        store_eng.dma_start(out=ys[rows, :], in_=yt[:])

                if i % 2 == 0:
                    continue

                p = i // 2
                if p in DIRECT_PAIRS:
                    # Column-sum both raw x tiles directly on the PE
                    # (8 matmuls instead of 4, but no DVE pair-add).
                    if i == 1:
                        group0 = [
                            (xh, c, 1) for c, xh in enumerate(xts[0])
                        ]
                    else:
                        group0 = [(xts[i - 1], 0, n_banks)]
                    for src, j0, nb in group0:
                        for jj in range(nb):
                            nc.tensor.matmul(
                                accs[j0 + jj][:],
                                ones[:],
                                src[:, jj * MM_N:(jj + 1) * MM_N],
                                start=(p == 0),
                                stop=False,
                            )
                    for j in range(n_banks):
                        nc.tensor.matmul(
                            accs[j][:],
                            ones[:],
                            xt[:, j * MM_N:(j + 1) * MM_N],
                            start=False,
                            stop=(p == N_TILES // 2 - 1),
                        )
                    del xts[i - 1], xts[i]
                    continue

                # Pair-add raw x tiles in f32, then PE column-sums z; this
                # halves the (4 cycles/row) fp32 matmul work vs summing
                # every tile while keeping the carry path fully fp32.
                z = zpool.tile([P, D], f32r)
                if i == 1:
                    for c, xh in enumerate(xts[0]):
                        cols = slice(c * (D // 4), (c + 1) * (D // 4))
                        nc.vector.tensor_add(
                            out=z[:, cols], in0=xh[:], in1=xt[:, cols]
                        )
                else:
                    nc.vector.tensor_add(out=z[:], in0=xts[i - 1][:], in1=xt[:])
                del xts[i - 1], xts[i]

                for j in range(n_banks):
                    nc.tensor.matmul(
                        accs[j][:],
                        ones_r[:],
                        z[:, j * MM_N:(j + 1) * MM_N],
                        start=(p == 0),
                        stop=False,
                    )

            res = cpool.tile([1, D], f32, tag="res", name="res")
            for j in range(n_banks):
                # PSUM -> SBUF on ACT as each bank's accumulation stops,
                # folding in the +1-bias contribution of the T_SHARD rows
                # this core summed.
                cols = slice(j * MM_N, (j + 1) * MM_N)
                nc.scalar.add(res[:, cols], accs[j][:], b_shard[:])
            # Single small store on the ACT queue (lightest at the tail).
            nc.scalar.dma_start(out=carry[:], in_=res[:])

    nc.compile()
    return nc


def kernel(xs):
    global _compiled_nc
    from concourse.bass_utils import run_bass_kernel_spmd

    xs = np.ascontiguousarray(np.asarray(xs, dtype=np.float32)).reshape(T, D)
    if _compiled_nc is None:
        _compiled_nc = _build_nc()

    in_maps = [{"x": xs[c * T_SHARD:(c + 1) * T_SHARD]} for c in range(N_CORES)]
    results = run_bass_kernel_spmd(_compiled_nc, in_maps, list(range(N_CORES))).results

    ys_full = np.concatenate([r["ys"] for r in results], axis=0)
    carry = np.zeros((1, D), dtype=np.float32)
    for r in results:
        carry += r["carry"]
    return ys_full, carry


# revision 56
# speedup vs baseline: 1.1527x; 1.0058x over previous
"""BiasAndSum Trainium2 kernel.

Reference semantics (xs: [T, 1, D] f32):
    ys    = xs[:, 0, :] + 1              # [T, D]
    carry = sum_t (xs[t] + 1)            # [1, D]

Strategy: shard T across 8 NeuronCores (T_SHARD = T/8 rows each). Per core,
stream 32 tiles of [128, D] with loads software-pipelined 6 tiles ahead:
  - DVE computes y = x + 1 in f32 (exact) for the ys store
  - the carry column-sum (a partition-axis reduction) runs on the PE as a
    ones[128, 1] stationary matmul into PSUM (fp32 accumulate); most pairs
    of x tiles are pre-added on DVE (halves the 4-cycles/row fp32 matmul
    work), a few early pairs are matmul'd directly so the PE ramps
    immediately, and the final pair is reduced in interleaved half tiles
    so the closing stop-matmuls drain alongside the stores; the +1-bias
    contribution (T_SHARD per column) is folded in when PSUM is read back
  - loads and stores are interleaved across the three DMA-capable issue
    queues (SP/ACT/Pool) so their transfer time is balanced
  - the first tile runs in quarter tiles and the last in half tiles to
    shorten pipeline fill/drain
Host concatenates ys shards and sums the 8 carry partials.
"""

import numpy as np

T, D = 32768, 2048
N_CORES = 8
T_SHARD = T // N_CORES  # 4096 rows per core
P = 128                 # SBUF partitions per tile
N_TILES = T_SHARD // P  # 32 tiles per core
MM_N = 512              # one PSUM bank of f32 per matmul output

# DMA issue queues are limited to SP (sync), Activation, and Pool (gpsimd).
# Per-DMA transfer time is charged to the issuing queue, so interleave both
# loads and stores across all three queues, slightly under-weighting ACT
# (which also runs two of the +1 ops): SP 22 / ACT 20 / Pool 22 DMAs.


def _weighted_rr(counts):
    """Spread engine indices with the given counts roughly uniformly."""
    total = sum(counts)
    credits = [0.0] * len(counts)
    out = []
    for _ in range(total):
        for k in range(len(counts)):
            credits[k] += counts[k] / total
        pick = max(range(len(counts)), key=lambda k: credits[k])
        credits[pick] -= 1.0
        out.append(pick)
    return out


LOAD_Q = _weighted_rr([11, 10, 11])   # SP, ACT, Pool per load tile
# Stores: tiles 10 and 20 are split in half across two queues so each queue
# carries ~21.5 MiB instead of 22 (the 64 transfers don't divide by 3); the
# remaining 30 stores rotate evenly.
SPLIT_STORE = {10: (1, 0), 20: (1, 2)}  # tile -> (engine for lo half, hi half)
_rot = _weighted_rr([10, 10, 10])
STORE_Q = []
_k = 0
for _i in range(32):
    if _i in SPLIT_STORE:
        STORE_Q.append(None)
    else:
        STORE_Q.append(_rot[_k])
        _k += 1
ACT_ADD = set()                       # tiles whose +1 runs on ACT instead of DVE
# Pairs whose column-sum skips the DVE pair-add and matmuls both x tiles
# directly on the PE (PE has slack; chosen at the start so PE ramps early).
DIRECT_PAIRS = {0, 1, 2}

_compiled_nc = None


def _build_nc():
    import concourse.bacc as bacc
    import concourse.mybir as mybir
    import concourse.tile as tile

    f32 = mybir.dt.float32

    nc = bacc.Bacc(
        "TRN2",
        target_bir_lowering=False,
        debug=False,
        enable_asserts=False,
        num_devices=N_CORES,
    )
    x = nc.dram_tensor("x", [T_SHARD, D], f32, kind="ExternalInput").ap()
    ys = nc.dram_tensor("ys", [T_SHARD, D], f32, kind="ExternalOutput").ap()
    carry = nc.dram_tensor("carry", [1, D], f32, kind="ExternalOutput").ap()

    n_banks = D // MM_N
    with tile.TileContext(nc) as tc:
        with (
            tc.tile_pool(name="x_pool", bufs=8) as xpool,
            tc.tile_pool(name="y_pool", bufs=8) as ypool,
            tc.tile_pool(name="z_pool", bufs=4) as zpool,
            tc.tile_pool(name="const_pool", bufs=1) as cpool,
            tc.tile_pool(name="psum_pool", bufs=1, space="PSUM") as ppool,
        ):
            ones = cpool.tile([P, 1], f32, tag="ones", name="ones")
            nc.vector.memset(ones[:], 1.0)
            # f32r twin for the pre-added z tiles: DVE produces z directly
            # as float32r, which runs the PE at 1 cycle/row instead of 4.
            f32r = mybir.dt.float32r
            ones_r = cpool.tile([P, 1], f32r, tag="ones_r", name="ones_r")
            nc.vector.memset(ones_r[:].bitcast(f32), 1.0)
            # Bias tile holding the +1 contribution of this core's T_SHARD
            # rows, applied when PSUM is read back on the scalar engine.
            b_shard = cpool.tile([1, 1], f32, tag="b_shard", name="b_shard")
            nc.vector.memset(b_shard[:], float(T_SHARD))

            accs = []
            for j in range(n_banks):
                acc = ppool.tile([1, MM_N], f32, tag=f"acc{j}", name=f"acc{j}")
                accs.append(acc)

            engines = [nc.sync, nc.scalar, nc.gpsimd]
            # Software-pipeline program order: issue the load for tile
            # i+LOOKAHEAD before tile i's compute+store, so each DMA queue's
            # FIFO keeps load supply ahead of the (non-gating) stores.
            LOOKAHEAD = 3
            xts = {}

            def issue_load(i):
                rows = slice(i * P, (i + 1) * P)
                if i == 0:
                    # Dependencies are tracked per tile, so the first tile's
                    # quarters must be separate tiles for the first +1 to
                    # start after only a quarter transfer (pipeline fill).
                    quarters = []
                    for c in range(4):
                        cols = slice(c * (D // 4), (c + 1) * (D // 4))
                        xh = xpool.tile(
                            [P, D // 4], f32, name=f"x0h{c}",
                            tag=f"x0h{c}", bufs=1,
                        )
                        engines[LOAD_Q[i]].dma_start(out=xh[:], in_=x[rows, cols])
                        quarters.append(xh)
                    xts[i] = quarters
                    return
                if i == N_TILES - 1:
                    # Last tile in separate half tiles so the closing
                    # reduction and +1 start after half a transfer
                    # (pipeline drain).
                    halves = []
                    for c in range(2):
                        cols = slice(c * (D // 2), (c + 1) * (D // 2))
                        xh = xpool.tile(
                            [P, D // 2], f32, name=f"xLh{c}",
                            tag=f"xLh{c}", bufs=1,
                        )
                        engines[LOAD_Q[i]].dma_start(out=xh[:], in_=x[rows, cols])
                        halves.append(xh)
                    xts[i] = halves
                    return
                if i == N_TILES - 1:
                    # Last tile in separate half tiles so the closing
                    # reduction and +1 start after half a transfer
                    # (pipeline drain).
                    halves = []
                    for c in range(2):
                        cols = slice(c * (D // 2), (c + 1) * (D // 2))
                        xh = xpool.tile(
                            [P, D // 2], f32, name=f"xLh{c}",
                            tag=f"xLh{c}", bufs=1,
                        )
                        engines[LOAD_Q[i]].dma_start(out=xh[:], in_=x[rows, cols])
                        halves.append(xh)
                    xts[i] = halves
                    return
                xt = xpool.tile([P, D], f32, name=f"xt{i}", tag="xt")
                engines[LOAD_Q[i]].dma_start(out=xt[:], in_=x[rows, :])
                xts[i] = xt

            for i in range(LOOKAHEAD):
                issue_load(i)

            for i in range(N_TILES):
                if i + LOOKAHEAD < N_TILES:
                    issue_load(i + LOOKAHEAD)
                rows = slice(i * P, (i + 1) * P)
                xt = xts[i]
                store_eng = None if STORE_Q[i] is None else engines[STORE_Q[i]]

                if i == 0:
                    # Separate quarter tiles (see issue_load): +1 and store
                    # per quarter so the pipeline fills as data lands.
                    for c, xh in enumerate(xt):
                        cols = slice(c * (D // 4), (c + 1) * (D // 4))
                        yh = ypool.tile(
                            [P, D // 4], f32, name=f"y0h{c}",
                            tag=f"y0h{c}", bufs=1,
                        )
                        nc.vector.tensor_scalar_add(
                            out=yh[:], in0=xh[:], scalar1=1.0
                        )
                        store_eng.dma_start(out=ys[rows, cols], in_=yh[:])
                    continue

                if i == N_TILES - 1:
                    # Drain stage: the last tile arrives as two half tiles;
                    # per half, DVE immediately pair-adds with the matching
                    # slice of the previous tile (feeding the closing
                    # stop-matmuls) while ACT computes the +1 for the store.
                    # Carry stays fully fp32.
                    for c, xh in enumerate(xt):
                        cols = slice(c * (D // 2), (c + 1) * (D // 2))
                        zh = zpool.tile(
                            [P, D // 2], f32r, name=f"zLh{c}",
                            tag=f"zLh{c}", bufs=1,
                        )
                        nc.vector.tensor_add(
                            out=zh[:], in0=xts[i - 1][:, cols], in1=xh[:]
                        )
                        yh = ypool.tile(
                            [P, D // 2], f32, name=f"yLh{c}",
                            tag=f"yLh{c}", bufs=1,
                        )
                        nc.scalar.add(yh[:], xh[:], 1.0)
                        store_eng.dma_start(out=ys[rows, cols], in_=yh[:])
                        for jj in range(n_banks // 2):
                            j = c * (n_banks // 2) + jj
                            nc.tensor.matmul(
                                accs[j][:],
                                ones_r[:],
                                zh[:, jj * MM_N:(jj + 1) * MM_N],
                                start=False,
                                stop=True,
                            )
                    del xts[i - 1], xts[i]
                    continue
                else:
                    yt = ypool.tile([P, D], f32)
                    # Exact f32 y = x + 1 for the ys store.
                    if i in ACT_ADD:
                        nc.scalar.add(yt[:], xt[:], 1.0)
                    else:
                        nc.vector.tensor_scalar_add(
                            out=yt[:], in0=xt[:], scalar1=1.0
                        )
                    if i in SPLIT_STORE:
                        e0, e1 = SPLIT_STORE[i]
                        h = D // 2
                        engines[e0].dma_start(
                            out=ys[rows, 0:h], in_=yt[:, 0:h]
                        )
                        engines[e1].dma_start(
                            out=ys[rows, h:D], in_=yt[:, h:D]
                        )
                    else:
                        store_eng.dma_start(out=ys[rows, :], in_=yt[:])

                if i % 2 == 0:
                    continue

                p = i // 2
                if p in DIRECT_PAIRS:
                    # Column-sum both raw x tiles directly on the PE
                    # (8 matmuls instead of 4, but no DVE pair-add).
                    if i == 1:
                        group0 = [
                            (xh, c, 1) for c, xh in enumerate(xts[0])
                        ]
                    else:
                        group0 = [(xts[i - 1], 0, n_banks)]
                    for src, j0, nb in group0:
                        for jj in range(nb):
                            nc.tensor.matmul(
                                accs[j0 + jj][:],
                                ones[:],
                                src[:, jj * MM_N:(jj + 1) * MM_N],
                                start=(p == 0),
                                stop=False,
                            )
                    for j in range(n_banks):
                        nc.tensor.matmul(
                            accs[j][:],
                            ones[:],
                            xt[:, j * MM_N:(j + 1) * MM_N],
                            start=False,
                            stop=(p == N_TILES // 2 - 1),
                        )
                    del xts[i - 1], xts[i]
                    continue

                # Pair-add raw x tiles in f32, then PE column-sums z; this
                # halves the (4 cycles/row) fp32 matmul work vs summing
                # every tile while keeping the carry path fully fp32.
                z = zpool.tile([P, D], f32r)
                if i == 1:
                    for c, xh in enumerate(xts[0]):
                        cols = slice(c * (D // 4), (c + 1) * (D // 4))
                        nc.vector.tensor_add(
                            out=z[:, cols], in0=xh[:], in1=xt[:, cols]
                        )
                else:
                    nc.vector.tensor_add(out=z[:], in0=xts[i - 1][:], in1=xt[:])
                del xts[i - 1], xts[i]

                for j in range(n_banks):
                    nc.tensor.matmul(
                        accs[j][:],
                        ones_r[:],
                        z[:, j * MM_N:(j + 1) * MM_N],
                        start=(p == 0),
                        stop=False,
                    )

            res = cpool.tile([1, D], f32, tag="res", name="res")
            for j in range(n_banks):
                # PSUM -> SBUF on ACT as each bank's accumulation stops,
                # folding in the +1-bias contribution of the T_SHARD rows
                # this core summed.
                cols = slice(j * MM_N, (j + 1) * MM_N)
                nc.scalar.add(res[:, cols], accs[j][:], b_shard[:])
            # Single small store on the ACT queue (lightest at the tail).
            nc.scalar.dma_start(out=carry[:], in_=res[:])

    nc.compile()
    return nc


def kernel(xs):
    global _compiled_nc
    from concourse.bass_utils import run_bass_kernel_spmd

    xs = np.ascontiguousarray(np.asarray(xs, dtype=np.float32)).reshape(T, D)
    if _compiled_nc is None:
        _compiled_nc = _build_nc()

    in_maps = [{"x": xs[c * T_SHARD:(c + 1) * T_SHARD]} for c in range(N_CORES)]
    results = run_bass_kernel_spmd(_compiled_nc, in_maps, list(range(N_CORES))).results

    ys_full = np.concatenate([r["ys"] for r in results], axis=0)
    carry = np.zeros((1, D), dtype=np.float32)
    for r in results:
        carry += r["carry"]
    return ys_full, carry
